# revision 43
# baseline (speedup 1.0000x reference)
"""CartesianDecomposedAttention Trainium2 kernel.

Complex-valued MHA (B=8, S=512, D=1024, H=16, Dh=64) decomposed into real
arithmetic, data-parallel over the batch dim across 8 NeuronCores (one batch
element per core, no collectives).

Per-core pipeline (batch b), layouts transposed so the contraction dim sits
on SBUF partitions:
  1. qkvT[e,s] = wqkvT.T @ xT for q,k rows (e-tiles of 128, two heads each)
     via 3-matmul Karatsuba complex products, RoPE fused into the PSUM
     combine; v computed in natural [s,e] layout so it is the AV stationary
     operand directly.
  2. scoresT[t,s] per head via K=64 matmuls; the two heads of an e-tile
     pair run CONCURRENTLY in the PE array (row tile_position 0/64,
     adjacent program order).  Softmax without max-subtraction: exp via the
     tanh identity exp(x) = 2/(1-tanh(x/2)) - 1, so the whole kernel uses
     ONE ACT table set (silu_and_others: tanh+sin+abs) - no table thrash.
     Phase rotors: ADD_RANGE_WRAP custom-DVE op wraps Im*scale into
     [-pi,pi], then sin via the Sin table and cos = sin(pi/2 - |x|).
  3. Denominator = colsum(exp) over t via ones-matmul; reciprocal via
     reciprocal_approx_fast, broadcast across partitions with gpsimd
     partition_broadcast (base-0 [64,512] tiles only - sliced-output
     broadcast is broken in the ucode); o~T = v.T @ W~ with the two heads
     packed via col tile_position, scaled by the reciprocal on eviction.
  4. Output projection back to [s, e] with woT (Karatsuba).

Matmul operands bf16 (fp32 PSUM accumulation); softmax/trig math fp32.
"""

import os
import sys

sys.path.insert(0, "/opt/trn_rl_repo")

import math

import ml_dtypes
import numpy as np

import concourse.bass as bass
import concourse.mybir as mybir
import concourse.tile as tile
from concourse import bacc
from concourse.dve_ops import ADD_RANGE_WRAP

BF16 = ml_dtypes.bfloat16

B, S, DM, H, DH = 8, 512, 1024, 16, 64
NK = DM // 128          # 8 contraction chunks of 128
NQK = 2 * DM // 128     # 16 q,k e-tiles of 128 (two heads each)
NP = H // 2             # 8 head pairs
NST = S // 128          # 4 s-tiles / t-tiles
SCALE = 1.0 / math.sqrt(DH)
TWO_PI = 2.0 * math.pi

MM_DT = mybir.dt.bfloat16
TAB_DT = mybir.dt.bfloat16  # rope cos/sin tables
F32 = mybir.dt.float32


def build_body(nc, tc, io):
    AF = mybir.ActivationFunctionType
    ALU = mybir.AluOpType
    V = nc.vector
    G = nc.gpsimd
    A = nc.scalar
    PE = nc.tensor

    wq = {c: io[f"wq_{c}"] for c in "ris"}
    wv = {c: io[f"wv_{c}"] for c in "ris"}
    out = io["out"]

    const = tc.alloc_tile_pool(name="const", bufs=1)
    psum = tc.alloc_tile_pool(name="ps", bufs=7, space="PSUM")
    psum_sm = tc.alloc_tile_pool(name="ps_small", bufs=1, space="PSUM")
    qk_pool = tc.alloc_tile_pool(name="qk", bufs=1)
    v_pool = tc.alloc_tile_pool(name="vp", bufs=1)
    o_pool = tc.alloc_tile_pool(name="op", bufs=1)
    work = tc.alloc_tile_pool(name="wk", bufs=1)
    ph12 = tc.alloc_tile_pool(name="ph12", bufs=1)

    # ---------------- x first (critical path) ----------------
    x_sb = {}
    for nm, eng in (("r", nc.sync), ("i", nc.gpsimd), ("s", nc.gpsimd)):
        t = ph12.tile([128, NK * S], MM_DT, name=f"x_{nm}_sb", tag=f"x{nm}",
                      bufs=1)
        eng.dma_start(t[:], io[f"x_{nm}"][:])
        x_sb[nm] = t

    # ---------------- constants ----------------
    cos_sb = const.tile([128, S], TAB_DT, name="cos_sb")
    sin_sb = const.tile([128, S], TAB_DT, name="sin_sb")
    cosq_sb = const.tile([128, S], TAB_DT, name="cosq_sb")
    sinq_sb = const.tile([128, S], TAB_DT, name="sinq_sb")
    nc.sync.dma_start(cos_sb[:], io["cos"][:])
    nc.sync.dma_start(sin_sb[:], io["sin"][:])
    nc.sync.dma_start(cosq_sb[:], io["cos_q"][:])
    nc.sync.dma_start(sinq_sb[:], io["sin_q"][:])
    ones_sb = const.tile([128, 1], MM_DT, name="ones_sb")
    V.memset(ones_sb[:], 1.0)
    b_hpi = const.tile([128, 1], F32, name="b_hpi")
    V.memset(b_hpi[:], math.pi / 2)
    b_one = const.tile([128, 1], F32, name="b_one")
    V.memset(b_one[:], 1.0)
    b_none = const.tile([128, 1], F32, name="b_none")
    V.memset(b_none[:], -1.0)

    def xch(nm, k):
        return x_sb[nm][:, k * S:(k + 1) * S]

    # ---------------- phase 1a: V in natural [s, e] layout ----------------
    v_r_sb = [v_pool.tile([128, DM], MM_DT, name=f"v_r_{st}", tag="v_r",
                          bufs=NST) for st in range(NST)]
    v_i_sb = [v_pool.tile([128, DM], MM_DT, name=f"v_i_{st}", tag="v_i",
                          bufs=NST) for st in range(NST)]
    v_n_sb = [v_pool.tile([128, DM], MM_DT, name=f"v_n_{st}", tag="v_n",
                          bufs=NST) for st in range(NST)]

    wv_cur = {}

    def load_wv(n):
        for nm in ("r", "i", "s"):
            t = ph12.tile([128, NK * 512], MM_DT, name=f"wv_{nm}_{n}",
                          tag=f"wv{nm}", bufs=1)
            nc.sync.dma_start(t[:], wv[nm][n])
            wv_cur[nm] = t

    def v_chunk(n, st):
        def wvch(nm, k):
            return wv_cur[nm][:, k * 512:(k + 1) * 512]

        nsl = slice(n * 512, (n + 1) * 512)
        ssl = slice(st * 128, (st + 1) * 128)
        p1 = psum.tile([128, 512], F32, name=f"vp1_{st}_{n}", tag="ps")
        p2 = psum.tile([128, 512], F32, name=f"vp2_{st}_{n}", tag="ps")
        p3 = psum.tile([128, 512], F32, name=f"vp3_{st}_{n}", tag="ps")
        for k in range(NK):
            PE.matmul(p1[:], xch("r", k)[:, ssl], wvch("r", k),
                      start=(k == 0), stop=(k == NK - 1))
        for k in range(NK):
            PE.matmul(p2[:], xch("i", k)[:, ssl], wvch("i", k),
                      start=(k == 0), stop=(k == NK - 1))
        for k in range(NK):
            PE.matmul(p3[:], xch("s", k)[:, ssl], wvch("s", k),
                      start=(k == 0), stop=(k == NK - 1))
        # v_r = p1 - p2 ; v_i = p3 - p1 - p2 ; v_n = -v_i
        # stage all three psums to bf16 via ACT so the DVE runs in 4x mode
        pc = work.tile([128, 512], MM_DT, name=f"vpc_{st}_{n}", tag="pc",
                       bufs=2)
        pd = work.tile([128, 512], MM_DT, name=f"vpd_{st}_{n}", tag="pd",
                       bufs=2)
        pe_ = work.tile([128, 512], MM_DT, name=f"vpe_{st}_{n}", tag="eb",
                       bufs=2)
        A.activation(pc[:], p1[:], AF.Copy)
        A.activation(pd[:], p2[:], AF.Copy)
        A.activation(pe_[:], p3[:], AF.Copy)
        V.tensor_sub(v_r_sb[st][:, nsl], pc[:], pd[:])
        V.tensor_sub(v_i_sb[st][:, nsl], pe_[:], pc[:])
        V.tensor_sub(v_i_sb[st][:, nsl], v_i_sb[st][:, nsl], pd[:])
        V.tensor_scalar_mul(v_n_sb[st][:, nsl], v_i_sb[st][:, nsl], -1.0)

    # ---------------- phase 1b: one rotated q or k e-tile ----------------
    def qk_etile(which, j):
        et = j if which == "q" else NK + j
        w_t = {}
        for nm in ("r", "i", "s"):
            w_t[nm] = ph12.tile([128, NK * 128], MM_DT,
                                name=f"w_{nm}_{which}{j}", tag=f"wst_{nm}",
                                bufs=2)
            nc.sync.dma_start(w_t[nm][:], wq[nm][et])
        p1 = psum.tile([128, 512], F32, name=f"qk1_{which}{j}", tag="ps")
        p2 = psum.tile([128, 512], F32, name=f"qk2_{which}{j}", tag="ps")
        p3 = psum.tile([128, 512], F32, name=f"qk3_{which}{j}", tag="ps")
        for k in range(NK):
            ksl = slice(k * 128, (k + 1) * 128)
            PE.matmul(p1[:], w_t["r"][:, ksl], xch("r", k), start=(k == 0),
                      stop=(k == NK - 1))
        for k in range(NK):
            ksl = slice(k * 128, (k + 1) * 128)
            PE.matmul(p2[:], w_t["i"][:, ksl], xch("i", k), start=(k == 0),
                      stop=(k == NK - 1))
        for k in range(NK):
            ksl = slice(k * 128, (k + 1) * 128)
            PE.matmul(p3[:], w_t["s"][:, ksl], xch("s", k), start=(k == 0),
                      stop=(k == NK - 1))
        # Ar = p1 - p2 ; Ai = p3 - p1 - p2, then RoPE (f32, DVE 2x): the
        # bf16 variant of this combine costs ~4e-3 extra rel_err
        ar = work.tile([128, 512], F32, name=f"ar_{which}{j}", tag="ar",
                       bufs=2)
        ai = work.tile([128, 512], F32, name=f"ai_{which}{j}", tag="ai",
                       bufs=2)
        pc = work.tile([128, 512], F32, name=f"pc_{which}{j}", tag="rt1q",
                       bufs=2)
        pd = work.tile([128, 512], F32, name=f"pd_{which}{j}", tag="rt2q",
                       bufs=2)
        A.activation(pc[:], p1[:], AF.Copy)
        A.activation(pd[:], p2[:], AF.Copy)
        V.tensor_sub(ar[:], pc[:], pd[:])
        V.tensor_sub(ai[:], p3[:], pc[:])
        V.tensor_sub(ai[:], ai[:], pd[:])
        t1 = work.tile([128, 512], F32, name=f"rt1_{which}{j}", tag="rt1q",
                       bufs=2)
        t2 = work.tile([128, 512], F32, name=f"rt2_{which}{j}", tag="rt2q",
                       bufs=2)
        rot_r = qk_pool.tile([128, S], MM_DT, name=f"{which}rot_r_{j}",
                             tag=f"{which}_rot_r", bufs=3)
        rot_i = qk_pool.tile([128, S], MM_DT, name=f"{which}rot_i_{j}",
                             tag=f"{which}_rot_i", bufs=3)
        c_t = cos_sb if which == "k" else cosq_sb
        s_t = sin_sb if which == "k" else sinq_sb
        V.tensor_mul(t1[:], ar[:], c_t[:])
        V.tensor_mul(t2[:], ai[:], s_t[:])
        V.tensor_sub(rot_r[:], t1[:], t2[:])
        V.tensor_mul(t1[:], ar[:], s_t[:])
        V.tensor_mul(t2[:], ai[:], c_t[:])
        V.tensor_add(rot_i[:], t1[:], t2[:])
        if which == "k":
            neg_i = qk_pool.tile([128, S], MM_DT, name=f"krot_n_{j}",
                                 tag="k_rot_n", bufs=3)
            V.tensor_scalar_mul(neg_i[:], rot_i[:], -1.0)
            return rot_r, rot_i, neg_i
        return rot_r, rot_i

    # ---------------- output accumulators ----------------
    o_r = [o_pool.tile([128, S], MM_DT, name=f"o_r_{j}", tag="o_r", bufs=NP)
           for j in range(NP)]
    o_i = [o_pool.tile([128, S], MM_DT, name=f"o_i_{j}", tag="o_i", bufs=NP)
           for j in range(NP)]
    o_s = [o_pool.tile([128, S], MM_DT, name=f"o_s_{j}", tag="o_s", bufs=NP)
           for j in range(NP)]

    # ---------------- phase 2: attention per head pair ----------------
    # Emission is software-pipelined: the AV stage of pair j-1 is emitted
    # after the scores stage of pair j, so the in-order PE stream never
    # stalls waiting for the softmax elementwise chain.
    def attention_scores(j, qr, qi, kr, ki, kn):
        """Heads 2j (partitions 0:64) and 2j+1 (64:128): scores, softmax
        numerator exp(Re) and phase rotors cos/sin(Im)."""
        Wr = {}
        Wi = {}
        ebs = {}
        ssc = 1.0 - 1e-6
        for half in range(2):
            h = 2 * j + half
            ebs[half] = work.tile([128, NST * 512], MM_DT, name=f"eb_{h}",
                                  tag="eb", bufs=2)
            Wr[half] = work.tile([128, NST * 512], MM_DT, name=f"Wr_{h}",
                                 tag="Wr", bufs=2)
            Wi[half] = work.tile([128, NST * 512], MM_DT, name=f"Wi_{h}",
                                 tag="Wi", bufs=2)
        for ch in range(2):
            t_c = {}
            s1_c = {}
            c1_c = {}
            ph_c = {}
            for half in range(2):
                h = 2 * j + half
                t_c[half] = work.tile([128, 1024], F32, name=f"t_{h}_{ch}",
                                      tag="t", bufs=2)
                ph_c[half] = work.tile([128, 1024], F32, name=f"ph_{h}_{ch}",
                                       tag="ph", bufs=1)
                s1_c[half] = work.tile([128, 1024], MM_DT,
                                       name=f"s1_{h}_{ch}", tag="s1", bufs=2)
                c1_c[half] = work.tile([128, 1024], MM_DT,
                                       name=f"c1_{h}_{ch}", tag="c1", bufs=2)
            for sub in range(2):
                tt = ch * 2 + sub
                tsl = slice(tt * 128, (tt + 1) * 128)
                csl = slice(sub * 512, (sub + 1) * 512)
                ps_re = {}
                ps_im = {}
                for half in range(2):
                    h = 2 * j + half
                    ps_re[half] = psum.tile([128, 512], F32,
                                            name=f"re_{h}_{tt}", tag="ps")
                    ps_im[half] = psum.tile([128, 512], F32,
                                            name=f"im_{h}_{tt}", tag="ps")
                # adjacent MMs in different row groups -> concurrent
                for half in range(2):
                    hsl = slice(half * 64, (half + 1) * 64)
                    tp = (half * 64, 0)
                    PE.matmul(ps_re[half][:], kr[hsl, tsl], qr[hsl, :],
                              start=True, stop=False, tile_position=tp)
                for half in range(2):
                    hsl = slice(half * 64, (half + 1) * 64)
                    tp = (half * 64, 0)
                    PE.matmul(ps_re[half][:], ki[hsl, tsl], qi[hsl, :],
                              start=False, stop=True, tile_position=tp)
                for half in range(2):
                    hsl = slice(half * 64, (half + 1) * 64)
                    tp = (half * 64, 0)
                    PE.matmul(ps_im[half][:], kr[hsl, tsl], qi[hsl, :],
                              start=True, stop=False, tile_position=tp)
                for half in range(2):
                    hsl = slice(half * 64, (half + 1) * 64)
                    tp = (half * 64, 0)
                    PE.matmul(ps_im[half][:], kn[hsl, tsl], qr[hsl, :],
                              start=False, stop=True, tile_position=tp)
                for half in range(2):
                    # t = tanh(Re/2); Re already has SCALE from q tables
                    A.activation(t_c[half][:, csl], ps_re[half][:], AF.Tanh,
                                 scale=0.5)
                    # wrap Im into [-pi, pi] straight from PSUM; trig runs
                    # once per [128,1024] chunk below
                    V._custom_dve(ADD_RANGE_WRAP,
                                  out=ph_c[half][:, csl],
                                  in0=ps_im[half][:], s0=0.0, s1=math.pi,
                                  imm2=TWO_PI)
            hs = slice(ch * 1024, (ch + 1) * 1024)
            for half in range(2):
                A.activation(s1_c[half][:], ph_c[half][:], AF.Sin, scale=ssc)
                # cos x = sin(pi/2 - |x|) for x in [-pi, pi]
                A.activation(ph_c[half][:], ph_c[half][:], AF.Abs, scale=1.0)
                A.activation(c1_c[half][:], ph_c[half][:], AF.Sin,
                             bias=b_hpi[:], scale=-1.0)
                t_t = t_c[half]
                # e = 2/(1-t) - 1  (== (1+t)/(1-t) == exp(2*atanh(t)))
                # 1-t on ACT (Identity, scale=-1, bias=1): DVE is saturated
                A.activation(t_t[:], t_t[:], AF.Identity, bias=b_one[:],
                             scale=-1.0)
                V.reciprocal_approx_fast(out=t_t[:], in_=t_t[:])
                # e = 2r - 1 on ACT (Identity, scale=2, bias=-1): DVE is the
                # window-gating engine (93% vs ACT 81%)
                A.activation(ebs[half][:, hs], t_t[:], AF.Identity,
                             bias=b_none[:], scale=2.0)
                V.tensor_mul(Wr[half][:, hs], ebs[half][:, hs], c1_c[half][:])
                V.tensor_mul(Wi[half][:, hs], ebs[half][:, hs], s1_c[half][:])
        return Wr, Wi, ebs

    def attention_av(j, Wr, Wi, ebs):
        recip = {}
        for half in range(2):
            h = 2 * j + half
            # softmax denominator: colsum of e over t (partition axis -> PE)
            ps_sum = psum_sm.tile([1, 512], F32, name=f"sum_{h}",
                                  tag="ps_sum")
            for tt in range(NST):
                ssl = slice(tt * 512, (tt + 1) * 512)
                PE.matmul(ps_sum[:], ones_sb[:], ebs[half][:, ssl],
                          start=(tt == 0), stop=(tt == NST - 1))
            rec_t = work.tile([1, 512], F32, name=f"rec_{h}", tag="rec",
                              bufs=2)
            V.reciprocal_approx_fast(out=rec_t[:], in_=ps_sum[:])
            recip[half] = rec_t

        # AV: o~T[dh, s] = sum_t v[t, dh].T @ W~T[t, s], heads col-packed
        ps_or = psum.tile([128, 512], F32, name=f"avr_{j}", tag="ps")
        ps_oi = psum.tile([128, 512], F32, name=f"avi_{j}", tag="ps")
        def hargs(half):
            h = 2 * j + half
            return (slice(h * DH, (h + 1) * DH),
                    slice(half * 64, (half + 1) * 64), (0, half * 64))

        for tt in range(NST):
            ssl = slice(tt * 512, (tt + 1) * 512)
            for half in range(2):
                esl, osl, tp = hargs(half)
                PE.matmul(ps_or[osl, :], v_r_sb[tt][:, esl], Wr[half][:, ssl],
                          start=(tt == 0), stop=False, tile_position=tp,
                          skip_group_check=True)
            for half in range(2):
                esl, osl, tp = hargs(half)
                PE.matmul(ps_oi[osl, :], v_r_sb[tt][:, esl], Wi[half][:, ssl],
                          start=(tt == 0), stop=False, tile_position=tp,
                          skip_group_check=True)
        for tt in range(NST):
            ssl = slice(tt * 512, (tt + 1) * 512)
            for half in range(2):
                esl, osl, tp = hargs(half)
                PE.matmul(ps_or[osl, :], v_n_sb[tt][:, esl], Wi[half][:, ssl],
                          start=False, stop=(tt == NST - 1), tile_position=tp,
                          skip_group_check=True)
            for half in range(2):
                esl, osl, tp = hargs(half)
                PE.matmul(ps_oi[osl, :], v_i_sb[tt][:, esl], Wr[half][:, ssl],
                          start=False, stop=(tt == NST - 1), tile_position=tp,
                          skip_group_check=True)
        # assemble the [128,512] reciprocal pair (sliced-output pbcast is
        # broken, so: base-0 pbcasts + one ACT copy into the upper half),
        # letting the evictions run at full DVE width
        rb_t = work.tile([128, 512], F32, name=f"rbp_{j}", tag="rb", bufs=1)
        rb1 = work.tile([64, 512], F32, name=f"rb1_{j}", tag="rb1", bufs=1)
        G.partition_broadcast(rb_t[0:64, :], recip[0][:])
        G.partition_broadcast(rb1[:], recip[1][:])
        A.activation(rb_t[64:128, :], rb1[:], AF.Copy)
        V.tensor_mul(o_r[j][:, :], ps_or[:, :], rb_t[:])
        V.tensor_mul(o_i[j][:, :], ps_oi[:, :], rb_t[:])
        V.tensor_add(o_s[j][:, :], o_r[j][:, :], o_i[j][:, :])

    # Pair 0's scores (and their long DVE/ACT chain) are emitted before the
    # PE-dense V phase, so the elementwise engines ramp while the PE churns
    # through the V matmuls.
    qk_tiles = {}
    qk_tiles[0] = (qk_etile("q", 0), qk_etile("k", 0))
    qk_tiles[1] = (qk_etile("q", 1), qk_etile("k", 1))
    (qr, qi), (kr, ki, kn) = qk_tiles.pop(0)
    pending = (0, attention_scores(0, qr, qi, kr, ki, kn))
    load_wv(0)
    for st in range(NST):
        v_chunk(0, st)
    load_wv(1)
    for j in range(1, NP):
        if j - 1 < NST:
            v_chunk(1, j - 1)
        if j + 1 < NP:
            qk_tiles[j + 1] = (qk_etile("q", j + 1), qk_etile("k", j + 1))
        (qr, qi), (kr, ki, kn) = qk_tiles.pop(j)
        sc = attention_scores(j, qr, qi, kr, ki, kn)
        attention_av(pending[0], *pending[1])
        pending = (j, sc)
    attention_av(pending[0], *pending[1])

    ph12.release()

    # ---------------- phase 3: output projection (Karatsuba) -------------
    wo_pool = tc.alloc_tile_pool(name="wo_pool", bufs=1)
    out_pool = tc.alloc_tile_pool(name="out_pool", bufs=1)
    wo_sb = {}
    for nm in ("r", "i", "s"):
        t = wo_pool.tile([128, NK * DM], MM_DT, name=f"wo_{nm}_sb",
                         tag=f"wo{nm}", bufs=1)
        nc.sync.dma_start(t[:], io[f"wo_{nm}"][:])
        wo_sb[nm] = t

    def woch(nm, k, n):
        off = k * DM + n * 512
        return wo_sb[nm][:, off:off + 512]


    for st in range(NST):
        ssl = slice(st * 128, (st + 1) * 128)
        for n in range(2):
            p1 = psum.tile([128, 512], F32, name=f"pj1_{st}_{n}", tag="ps")
            p2 = psum.tile([128, 512], F32, name=f"pj2_{st}_{n}", tag="ps")
            p3 = psum.tile([128, 512], F32, name=f"pj3_{st}_{n}", tag="ps")
            for k in range(NK):
                PE.matmul(p1[:], o_r[k][:, ssl], woch("r", k, n),
                          start=(k == 0), stop=(k == NK - 1))
            for k in range(NK):
                PE.matmul(p2[:], o_i[k][:, ssl], woch("i", k, n),
                          start=(k == 0), stop=(k == NK - 1))
            for k in range(NK):
                PE.matmul(p3[:], o_s[k][:, ssl], woch("s", k, n),
                          start=(k == 0), stop=(k == NK - 1))
            to_r = out_pool.tile([128, 512], F32, name=f"otr_{st}_{n}",
                                 tag="out_r", bufs=2)
            to_i = out_pool.tile([128, 512], F32, name=f"oti_{st}_{n}",
                                 tag="out_i", bufs=2)
            ot = out_pool.tile([128, 512], F32, name=f"ott_{st}_{n}",
                               tag="out_t", bufs=2)
            A.activation(ot[:], p1[:], AF.Copy)
            V.tensor_sub(to_r[:], ot[:], p2[:])
            V.tensor_sub(to_i[:], p3[:], ot[:])
            V.tensor_sub(to_i[:], to_i[:], p2[:])
            nsl = slice(n * 512, (n + 1) * 512)
            nc.sync.dma_start(out[0, ssl, nsl], to_r[:])
            nc.sync.dma_start(out[1, ssl, nsl], to_i[:])

    out_pool.release()
    wo_pool.release()
    for p in (work, o_pool, v_pool, qk_pool, psum_sm, psum, const):
        p.release()


def _install_act_root():
    """Restrict walrus to the silu_and_others ACT table set (tanh+sin+abs
    +copy in ONE set) so the kernel never thrashes ACT_TABLE_LOADs.
    On any failure, degrade to the default tables (correct, slower)."""
    if os.environ.get("K_NO_ACTFIX"):
        return
    if os.environ.get("BASS_ACT_ROOT_JSON_PATH"):
        return
    try:
        _install_act_root_impl()
    except Exception:
        os.environ["K_NO_ACTFIX"] = "1"


def _install_act_root_impl():
    import json
    import tempfile
    from neuronxcc.driver.Job import Job
    from neuronxcc.driver.jobs.support.FindActInfo import findActInfoFile

    p = findActInfoFile(Job.getPackageDir(), "gen3")
    src_dir = os.path.dirname(p)
    with open(p) as f:
        d = json.load(f)
    d["act_func_sets"] = [e for e in d["act_func_sets"]
                          if e["name"] == "silu_and_others"]
    out_dir = tempfile.mkdtemp(prefix="act_silu_")
    for fn in os.listdir(src_dir):
        sp = os.path.join(src_dir, fn)
        if os.path.isfile(sp) and fn != "act_info.json":
            os.symlink(sp, os.path.join(out_dir, fn))
    with open(os.path.join(out_dir, "act_info.json"), "w") as f:
        json.dump(d, f)
    os.environ["BASS_ACT_ROOT_JSON_PATH"] = os.path.join(out_dir,
                                                         "act_info.json")
    # bass pre-places InstLoadActFuncSet ids indexing this same list; keep
    # the bass-side table view consistent with the trimmed act_info.json.
    import concourse.hw_specs as hw_specs
    import concourse.bacc as bacc_mod

    orig = hw_specs.get_activation_tables.__wrapped__

    @__import__("functools").cache
    def only_silu(arch):
        full = orig(arch)
        return {"silu_and_others": full["silu_and_others"]}

    hw_specs.get_activation_tables = only_silu
    bacc_mod.get_activation_tables = only_silu


def build_nc():
    _install_act_root()
    nc = bacc.Bacc("TRN2", target_bir_lowering=False, debug=False,
                   enable_asserts=False, num_devices=8)
    io = {}

    def inp(name, shape, dt=MM_DT):
        io[name] = nc.dram_tensor(name, shape, dt, kind="ExternalInput").ap()

    inp("x_r", [128, NK * S])
    inp("x_i", [128, NK * S])
    inp("x_s", [128, NK * S])
    inp("wq_r", [NQK, 128, NK * 128])
    inp("wq_i", [NQK, 128, NK * 128])
    inp("wq_s", [NQK, 128, NK * 128])
    inp("wv_r", [2, 128, NK * 512])
    inp("wv_i", [2, 128, NK * 512])
    inp("wv_s", [2, 128, NK * 512])
    inp("wo_r", [128, NK * DM])
    inp("wo_i", [128, NK * DM])
    inp("wo_s", [128, NK * DM])
    inp("cos", [128, S], TAB_DT)
    inp("sin", [128, S], TAB_DT)
    inp("cos_q", [128, S], TAB_DT)
    inp("sin_q", [128, S], TAB_DT)
    io["out"] = nc.dram_tensor("out", [2, S, DM], F32,
                               kind="ExternalOutput").ap()

    with tile.TileContext(nc) as tc:
        build_body(nc, tc, io)
    nc.compile()
    return nc


def host_inputs(xr, xi, wqkv_r, wqkv_i, wo_r, wo_i):
    """Pack full f32 inputs into 8 per-core in_maps."""
    np_mm = mybir.dt.np(MM_DT)
    np_tab = mybir.dt.np(TAB_DT)

    def pack_qk(w):  # (D, 3D) -> [16e][128p][8k*128]
        return np.ascontiguousarray(
            w[:, :2 * DM].reshape(NK, 128, NQK, 128).transpose(2, 1, 0, 3)
            .reshape(NQK, 128, NK * 128))

    def pack_v(w):  # -> [2n][128p][8k*512]
        return np.ascontiguousarray(
            w[:, 2 * DM:].reshape(NK, 128, 2, 512).transpose(2, 1, 0, 3)
            .reshape(2, 128, NK * 512))

    def pack_p(w):  # (NK,128,F) row-major -> [128p][NK*F]
        return np.ascontiguousarray(
            w.transpose(1, 0, 2).reshape(128, -1))

    wqkvT_r = np.ascontiguousarray(wqkv_r.T).astype(np_mm)  # (D, 3D)
    wqkvT_i = np.ascontiguousarray(wqkv_i.T).astype(np_mm)
    wqkvT_s = (wqkvT_r.astype(np.float32)
               + wqkvT_i.astype(np.float32)).astype(np_mm)
    woT_r = np.ascontiguousarray(wo_r.T.astype(np_mm))
    woT_i = np.ascontiguousarray(wo_i.T.astype(np_mm))
    woT_s = (woT_r.astype(np.float32)
             + woT_i.astype(np.float32)).astype(np_mm)

    inv_freq = 1.0 / (10000.0 ** (np.arange(DH, dtype=np.float64) / DH))
    ang = np.arange(S, dtype=np.float64)[:, None] * inv_freq[None, :]  # (S,Dh)
    cosT = np.cos(ang).T  # (Dh, S)
    sinT = np.sin(ang).T
    cos_t = np.ascontiguousarray(
        np.concatenate([cosT, cosT], axis=0)).astype(np_tab)  # (128, S)
    sin_t = np.ascontiguousarray(
        np.concatenate([sinT, sinT], axis=0)).astype(np_tab)

    shared = {
        "wq_r": pack_qk(wqkvT_r), "wq_i": pack_qk(wqkvT_i),
        "wq_s": pack_qk(wqkvT_s),
        "wv_r": pack_v(wqkvT_r), "wv_i": pack_v(wqkvT_i),
        "wv_s": pack_v(wqkvT_s),
        "wo_r": pack_p(woT_r.reshape(NK, 128, DM)),
        "wo_i": pack_p(woT_i.reshape(NK, 128, DM)),
        "wo_s": pack_p(woT_s.reshape(NK, 128, DM)),
        "cos": cos_t, "sin": sin_t,
        "cos_q": np.ascontiguousarray(
            np.concatenate([cosT, cosT], axis=0) * SCALE).astype(np_tab),
        "sin_q": np.ascontiguousarray(
            np.concatenate([sinT, sinT], axis=0) * SCALE).astype(np_tab),
    }
    in_maps = []
    for b in range(B):
        xT_r = xr[b].T.astype(np_mm).reshape(NK, 128, S)
        xT_i = xi[b].T.astype(np_mm).reshape(NK, 128, S)
        xT_s = (xT_r.astype(np.float32)
                + xT_i.astype(np.float32)).astype(np_mm)
        m = {"x_r": pack_p(xT_r), "x_i": pack_p(xT_i), "x_s": pack_p(xT_s)}
        m.update(shared)
        in_maps.append(m)
    return in_maps


_NC_CACHE = None


def get_nc():
    global _NC_CACHE
    if _NC_CACHE is None:
        _NC_CACHE = build_nc()
    return _NC_CACHE


def kernel(xr, xi, wqkv_r, wqkv_i, wo_r, wo_i):
    from concourse.bass_utils import run_bass_kernel_spmd

    _install_act_root()
    in_maps = host_inputs(np.asarray(xr, np.float32),
                          np.asarray(xi, np.float32),
                          np.asarray(wqkv_r, np.float32),
                          np.asarray(wqkv_i, np.float32),
                          np.asarray(wo_r, np.float32),
                          np.asarray(wo_i, np.float32))
    nc = get_nc()
    res = run_bass_kernel_spmd(nc, in_maps, core_ids=list(range(B)),
                               trace=bool(int(os.environ.get("K_TRACE", "0"))))
    out_r = np.stack([res.results[b]["out"][0] for b in range(B)])
    out_i = np.stack([res.results[b]["out"][1] for b in range(B)])
    kernel.last_results = res
    return out_r, out_i


# revision 44
# speedup vs baseline: 1.0339x; 1.0339x over previous
"""CartesianDecomposedAttention Trainium2 kernel.

Complex-valued MHA (B=8, S=512, D=1024, H=16, Dh=64) decomposed into real
arithmetic, data-parallel over the batch dim across 8 NeuronCores (one batch
element per core, no collectives).

Per-core pipeline (batch b), layouts transposed so the contraction dim sits
on SBUF partitions:
  1. qkvT[e,s] = wqkvT.T @ xT for q,k rows (e-tiles of 128, two heads each)
     via 3-matmul Karatsuba complex products, RoPE fused into the PSUM
     combine; v computed in natural [s,e] layout so it is the AV stationary
     operand directly.
  2. scoresT[t,s] per head via K=64 matmuls; the two heads of an e-tile
     pair run CONCURRENTLY in the PE array (row tile_position 0/64,
     adjacent program order).  Softmax without max-subtraction: exp via the
     tanh identity exp(x) = 2/(1-tanh(x/2)) - 1, so the whole kernel uses
     ONE ACT table set (silu_and_others: tanh+sin+abs) - no table thrash.
     Phase rotors: ADD_RANGE_WRAP custom-DVE op wraps Im*scale into
     [-pi,pi], then sin via the Sin table and cos = sin(pi/2 - |x|).
  3. Denominator = colsum(exp) over t via ones-matmul; reciprocal via
     reciprocal_approx_fast, broadcast across partitions with gpsimd
     partition_broadcast (base-0 [64,512] tiles only - sliced-output
     broadcast is broken in the ucode); o~T = v.T @ W~ with the two heads
     packed via col tile_position, scaled by the reciprocal on eviction.
  4. Output projection back to [s, e] with woT (Karatsuba).

Matmul operands bf16 (fp32 PSUM accumulation); softmax/trig math fp32.
"""

import os
import sys

sys.path.insert(0, "/opt/trn_rl_repo")

import math

import ml_dtypes
import numpy as np

import concourse.bass as bass
import concourse.mybir as mybir
import concourse.tile as tile
from concourse import bacc
from concourse.dve_ops import ADD_RANGE_WRAP

BF16 = ml_dtypes.bfloat16

B, S, DM, H, DH = 8, 512, 1024, 16, 64
NK = DM // 128          # 8 contraction chunks of 128
NQK = 2 * DM // 128     # 16 q,k e-tiles of 128 (two heads each)
NP = H // 2             # 8 head pairs
NST = S // 128          # 4 s-tiles / t-tiles
SCALE = 1.0 / math.sqrt(DH)
TWO_PI = 2.0 * math.pi

MM_DT = mybir.dt.bfloat16
TAB_DT = mybir.dt.bfloat16  # rope cos/sin tables
F32 = mybir.dt.float32


def build_body(nc, tc, io):
    AF = mybir.ActivationFunctionType
    ALU = mybir.AluOpType
    V = nc.vector
    G = nc.gpsimd
    A = nc.scalar
    PE = nc.tensor

    wq = {c: io[f"wq_{c}"] for c in "ris"}
    wv = {c: io[f"wv_{c}"] for c in "ris"}
    out = io["out"]

    const = tc.alloc_tile_pool(name="const", bufs=1)
    psum = tc.alloc_tile_pool(name="ps", bufs=7, space="PSUM")
    psum_sm = tc.alloc_tile_pool(name="ps_small", bufs=1, space="PSUM")
    qk_pool = tc.alloc_tile_pool(name="qk", bufs=1)
    v_pool = tc.alloc_tile_pool(name="vp", bufs=1)
    o_pool = tc.alloc_tile_pool(name="op", bufs=1)
    work = tc.alloc_tile_pool(name="wk", bufs=1)
    ph12 = tc.alloc_tile_pool(name="ph12", bufs=1)

    # ---------------- x first (critical path) ----------------
    x_sb = {}
    for nm, eng in (("r", nc.sync), ("i", nc.gpsimd), ("s", nc.gpsimd)):
        t = ph12.tile([128, NK * S], MM_DT, name=f"x_{nm}_sb", tag=f"x{nm}",
                      bufs=1)
        eng.dma_start(t[:], io[f"x_{nm}"][:])
        x_sb[nm] = t

    # ---------------- constants ----------------
    cos_sb = const.tile([128, S], TAB_DT, name="cos_sb")
    sin_sb = const.tile([128, S], TAB_DT, name="sin_sb")
    cosq_sb = const.tile([128, S], TAB_DT, name="cosq_sb")
    sinq_sb = const.tile([128, S], TAB_DT, name="sinq_sb")
    nc.sync.dma_start(cos_sb[:], io["cos"][:])
    nc.sync.dma_start(sin_sb[:], io["sin"][:])
    nc.sync.dma_start(cosq_sb[:], io["cos_q"][:])
    nc.sync.dma_start(sinq_sb[:], io["sin_q"][:])
    ones_sb = const.tile([128, 1], MM_DT, name="ones_sb")
    V.memset(ones_sb[:], 1.0)
    b_hpi = const.tile([128, 1], F32, name="b_hpi")
    V.memset(b_hpi[:], math.pi / 2)
    b_one = const.tile([128, 1], F32, name="b_one")
    V.memset(b_one[:], 1.0)

    def xch(nm, k):
        return x_sb[nm][:, k * S:(k + 1) * S]

    # ---------------- phase 1a: V in natural [s, e] layout ----------------
    v_r_sb = [v_pool.tile([128, DM], MM_DT, name=f"v_r_{st}", tag="v_r",
                          bufs=NST) for st in range(NST)]
    v_i_sb = [v_pool.tile([128, DM], MM_DT, name=f"v_i_{st}", tag="v_i",
                          bufs=NST) for st in range(NST)]
    v_n_sb = [v_pool.tile([128, DM], MM_DT, name=f"v_n_{st}", tag="v_n",
                          bufs=NST) for st in range(NST)]

    wv_cur = {}

    def load_wv(n):
        for nm in ("r", "i", "s"):
            t = ph12.tile([128, NK * 512], MM_DT, name=f"wv_{nm}_{n}",
                          tag=f"wv{nm}", bufs=1)
            nc.sync.dma_start(t[:], wv[nm][n])
            wv_cur[nm] = t

    def v_chunk(n, st):
        def wvch(nm, k):
            return wv_cur[nm][:, k * 512:(k + 1) * 512]

        nsl = slice(n * 512, (n + 1) * 512)
        ssl = slice(st * 128, (st + 1) * 128)
        p1 = psum.tile([128, 512], F32, name=f"vp1_{st}_{n}", tag="ps")
        p2 = psum.tile([128, 512], F32, name=f"vp2_{st}_{n}", tag="ps")
        p3 = psum.tile([128, 512], F32, name=f"vp3_{st}_{n}", tag="ps")
        for k in range(NK):
            PE.matmul(p1[:], xch("r", k)[:, ssl], wvch("r", k),
                      start=(k == 0), stop=(k == NK - 1))
        for k in range(NK):
            PE.matmul(p2[:], xch("i", k)[:, ssl], wvch("i", k),
                      start=(k == 0), stop=(k == NK - 1))
        for k in range(NK):
            PE.matmul(p3[:], xch("s", k)[:, ssl], wvch("s", k),
                      start=(k == 0), stop=(k == NK - 1))
        # v_r = p1 - p2 ; v_i = p3 - p1 - p2 ; v_n = -v_i
        # stage all three psums to bf16 via ACT so the DVE runs in 4x mode
        pc = work.tile([128, 512], MM_DT, name=f"vpc_{st}_{n}", tag="pc",
                       bufs=2)
        pd = work.tile([128, 512], MM_DT, name=f"vpd_{st}_{n}", tag="pd",
                       bufs=2)
        pe_ = work.tile([128, 512], MM_DT, name=f"vpe_{st}_{n}", tag="eb",
                       bufs=2)
        A.activation(pc[:], p1[:], AF.Copy)
        A.activation(pd[:], p2[:], AF.Copy)
        A.activation(pe_[:], p3[:], AF.Copy)
        V.tensor_sub(v_r_sb[st][:, nsl], pc[:], pd[:])
        V.tensor_sub(v_i_sb[st][:, nsl], pe_[:], pc[:])
        V.tensor_sub(v_i_sb[st][:, nsl], v_i_sb[st][:, nsl], pd[:])
        V.tensor_scalar_mul(v_n_sb[st][:, nsl], v_i_sb[st][:, nsl], -1.0)

    # ---------------- phase 1b: one rotated q or k e-tile ----------------
    def qk_etile(which, j):
        et = j if which == "q" else NK + j
        w_t = {}
        for nm in ("r", "i", "s"):
            w_t[nm] = ph12.tile([128, NK * 128], MM_DT,
                                name=f"w_{nm}_{which}{j}", tag=f"wst_{nm}",
                                bufs=2)
            nc.sync.dma_start(w_t[nm][:], wq[nm][et])
        p1 = psum.tile([128, 512], F32, name=f"qk1_{which}{j}", tag="ps")
        p2 = psum.tile([128, 512], F32, name=f"qk2_{which}{j}", tag="ps")
        p3 = psum.tile([128, 512], F32, name=f"qk3_{which}{j}", tag="ps")
        for k in range(NK):
            ksl = slice(k * 128, (k + 1) * 128)
            PE.matmul(p1[:], w_t["r"][:, ksl], xch("r", k), start=(k == 0),
                      stop=(k == NK - 1))
        for k in range(NK):
            ksl = slice(k * 128, (k + 1) * 128)
            PE.matmul(p2[:], w_t["i"][:, ksl], xch("i", k), start=(k == 0),
                      stop=(k == NK - 1))
        for k in range(NK):
            ksl = slice(k * 128, (k + 1) * 128)
            PE.matmul(p3[:], w_t["s"][:, ksl], xch("s", k), start=(k == 0),
                      stop=(k == NK - 1))
        # Ar = p1 - p2 ; Ai = p3 - p1 - p2, then RoPE (f32, DVE 2x): the
        # bf16 variant of this combine costs ~4e-3 extra rel_err
        ar = work.tile([128, 512], F32, name=f"ar_{which}{j}", tag="ar",
                       bufs=2)
        ai = work.tile([128, 512], F32, name=f"ai_{which}{j}", tag="ai",
                       bufs=2)
        pc = work.tile([128, 512], F32, name=f"pc_{which}{j}", tag="rt1q",
                       bufs=2)
        pd = work.tile([128, 512], F32, name=f"pd_{which}{j}", tag="rt2q",
                       bufs=2)
        A.activation(pc[:], p1[:], AF.Copy)
        A.activation(pd[:], p2[:], AF.Copy)
        V.tensor_sub(ar[:], pc[:], pd[:])
        V.tensor_sub(ai[:], p3[:], pc[:])
        V.tensor_sub(ai[:], ai[:], pd[:])
        t1 = work.tile([128, 512], F32, name=f"rt1_{which}{j}", tag="rt1q",
                       bufs=2)
        t2 = work.tile([128, 512], F32, name=f"rt2_{which}{j}", tag="rt2q",
                       bufs=2)
        rot_r = qk_pool.tile([128, S], MM_DT, name=f"{which}rot_r_{j}",
                             tag=f"{which}_rot_r", bufs=3)
        rot_i = qk_pool.tile([128, S], MM_DT, name=f"{which}rot_i_{j}",
                             tag=f"{which}_rot_i", bufs=3)
        c_t = cos_sb if which == "k" else cosq_sb
        s_t = sin_sb if which == "k" else sinq_sb
        V.tensor_mul(t1[:], ar[:], c_t[:])
        V.tensor_mul(t2[:], ai[:], s_t[:])
        V.tensor_sub(rot_r[:], t1[:], t2[:])
        V.tensor_mul(t1[:], ar[:], s_t[:])
        V.tensor_mul(t2[:], ai[:], c_t[:])
        V.tensor_add(rot_i[:], t1[:], t2[:])
        if which == "k":
            neg_i = qk_pool.tile([128, S], MM_DT, name=f"krot_n_{j}",
                                 tag="k_rot_n", bufs=3)
            V.tensor_scalar_mul(neg_i[:], rot_i[:], -1.0)
            return rot_r, rot_i, neg_i
        return rot_r, rot_i

    # ---------------- output accumulators ----------------
    o_r = [o_pool.tile([128, S], MM_DT, name=f"o_r_{j}", tag="o_r", bufs=NP)
           for j in range(NP)]
    o_i = [o_pool.tile([128, S], MM_DT, name=f"o_i_{j}", tag="o_i", bufs=NP)
           for j in range(NP)]
    o_s = [o_pool.tile([128, S], MM_DT, name=f"o_s_{j}", tag="o_s", bufs=NP)
           for j in range(NP)]

    # ---------------- phase 2: attention per head pair ----------------
    # Emission is software-pipelined: the AV stage of pair j-1 is emitted
    # after the scores stage of pair j, so the in-order PE stream never
    # stalls waiting for the softmax elementwise chain.
    def attention_scores(j, qr, qi, kr, ki, kn):
        """Heads 2j (partitions 0:64) and 2j+1 (64:128): scores, softmax
        numerator exp(Re) and phase rotors cos/sin(Im)."""
        Wr = {}
        Wi = {}
        ebs = {}
        ssc = 1.0 - 1e-6
        for half in range(2):
            h = 2 * j + half
            ebs[half] = work.tile([128, NST * 512], MM_DT, name=f"eb_{h}",
                                  tag="eb", bufs=2)
            Wr[half] = work.tile([128, NST * 512], MM_DT, name=f"Wr_{h}",
                                 tag="Wr", bufs=2)
            Wi[half] = work.tile([128, NST * 512], MM_DT, name=f"Wi_{h}",
                                 tag="Wi", bufs=2)
        for ch in range(2):
            t_c = {}
            s1_c = {}
            c1_c = {}
            ph_c = {}
            for half in range(2):
                h = 2 * j + half
                t_c[half] = work.tile([128, 1024], F32, name=f"t_{h}_{ch}",
                                      tag="t", bufs=2)
                ph_c[half] = work.tile([128, 1024], F32, name=f"ph_{h}_{ch}",
                                       tag="ph", bufs=1)
                s1_c[half] = work.tile([128, 1024], MM_DT,
                                       name=f"s1_{h}_{ch}", tag="s1", bufs=2)
                c1_c[half] = work.tile([128, 1024], MM_DT,
                                       name=f"c1_{h}_{ch}", tag="c1", bufs=2)
            for sub in range(2):
                tt = ch * 2 + sub
                tsl = slice(tt * 128, (tt + 1) * 128)
                csl = slice(sub * 512, (sub + 1) * 512)
                ps_re = {}
                ps_im = {}
                for half in range(2):
                    h = 2 * j + half
                    ps_re[half] = psum.tile([128, 512], F32,
                                            name=f"re_{h}_{tt}", tag="ps")
                    ps_im[half] = psum.tile([128, 512], F32,
                                            name=f"im_{h}_{tt}", tag="ps")
                # adjacent MMs in different row groups -> concurrent
                for half in range(2):
                    hsl = slice(half * 64, (half + 1) * 64)
                    tp = (half * 64, 0)
                    PE.matmul(ps_re[half][:], kr[hsl, tsl], qr[hsl, :],
                              start=True, stop=False, tile_position=tp)
                for half in range(2):
                    hsl = slice(half * 64, (half + 1) * 64)
                    tp = (half * 64, 0)
                    PE.matmul(ps_re[half][:], ki[hsl, tsl], qi[hsl, :],
                              start=False, stop=True, tile_position=tp)
                for half in range(2):
                    hsl = slice(half * 64, (half + 1) * 64)
                    tp = (half * 64, 0)
                    PE.matmul(ps_im[half][:], kr[hsl, tsl], qi[hsl, :],
                              start=True, stop=False, tile_position=tp)
                for half in range(2):
                    hsl = slice(half * 64, (half + 1) * 64)
                    tp = (half * 64, 0)
                    PE.matmul(ps_im[half][:], kn[hsl, tsl], qr[hsl, :],
                              start=False, stop=True, tile_position=tp)
                for half in range(2):
                    # t = tanh(Re/2); Re already has SCALE from q tables
                    A.activation(t_c[half][:, csl], ps_re[half][:], AF.Tanh,
                                 scale=0.5)
                    # wrap Im into [-pi, pi] straight from PSUM; trig runs
                    # once per [128,1024] chunk below
                    V._custom_dve(ADD_RANGE_WRAP,
                                  out=ph_c[half][:, csl],
                                  in0=ps_im[half][:], s0=0.0, s1=math.pi,
                                  imm2=TWO_PI)
            hs = slice(ch * 1024, (ch + 1) * 1024)
            for half in range(2):
                A.activation(s1_c[half][:], ph_c[half][:], AF.Sin, scale=ssc)
                # cos x = sin(pi/2 - |x|) for x in [-pi, pi]
                A.activation(ph_c[half][:], ph_c[half][:], AF.Abs, scale=1.0)
                A.activation(c1_c[half][:], ph_c[half][:], AF.Sin,
                             bias=b_hpi[:], scale=-1.0)
                t_t = t_c[half]
                # e = 2/(1-t) - 1  (== (1+t)/(1-t) == exp(2*atanh(t)))
                # 1-t on ACT (Identity, scale=-1, bias=1): DVE is saturated
                A.activation(t_t[:], t_t[:], AF.Identity, bias=b_one[:],
                             scale=-1.0)
                V.reciprocal_approx_fast(out=t_t[:], in_=t_t[:])
                V.tensor_scalar(ebs[half][:, hs], t_t[:], 2.0, -1.0,
                                ALU.mult, ALU.add)
                V.tensor_mul(Wr[half][:, hs], ebs[half][:, hs], c1_c[half][:])
                V.tensor_mul(Wi[half][:, hs], ebs[half][:, hs], s1_c[half][:])
        return Wr, Wi, ebs

    def attention_av(j, Wr, Wi, ebs):
        recip = {}
        for half in range(2):
            h = 2 * j + half
            # softmax denominator: colsum of e over t (partition axis -> PE)
            ps_sum = psum_sm.tile([1, 512], F32, name=f"sum_{h}",
                                  tag="ps_sum")
            for tt in range(NST):
                ssl = slice(tt * 512, (tt + 1) * 512)
                PE.matmul(ps_sum[:], ones_sb[:], ebs[half][:, ssl],
                          start=(tt == 0), stop=(tt == NST - 1))
            rec_t = work.tile([1, 512], F32, name=f"rec_{h}", tag="rec",
                              bufs=2)
            V.reciprocal_approx_fast(out=rec_t[:], in_=ps_sum[:])
            recip[half] = rec_t

        # AV: o~T[dh, s] = sum_t v[t, dh].T @ W~T[t, s], heads col-packed
        ps_or = psum.tile([128, 512], F32, name=f"avr_{j}", tag="ps")
        ps_oi = psum.tile([128, 512], F32, name=f"avi_{j}", tag="ps")
        def hargs(half):
            h = 2 * j + half
            return (slice(h * DH, (h + 1) * DH),
                    slice(half * 64, (half + 1) * 64), (0, half * 64))

        for tt in range(NST):
            ssl = slice(tt * 512, (tt + 1) * 512)
            for half in range(2):
                esl, osl, tp = hargs(half)
                PE.matmul(ps_or[osl, :], v_r_sb[tt][:, esl], Wr[half][:, ssl],
                          start=(tt == 0), stop=False, tile_position=tp,
                          skip_group_check=True)
            for half in range(2):
                esl, osl, tp = hargs(half)
                PE.matmul(ps_oi[osl, :], v_r_sb[tt][:, esl], Wi[half][:, ssl],
                          start=(tt == 0), stop=False, tile_position=tp,
                          skip_group_check=True)
        for tt in range(NST):
            ssl = slice(tt * 512, (tt + 1) * 512)
            for half in range(2):
                esl, osl, tp = hargs(half)
                PE.matmul(ps_or[osl, :], v_n_sb[tt][:, esl], Wi[half][:, ssl],
                          start=False, stop=(tt == NST - 1), tile_position=tp,
                          skip_group_check=True)
            for half in range(2):
                esl, osl, tp = hargs(half)
                PE.matmul(ps_oi[osl, :], v_i_sb[tt][:, esl], Wr[half][:, ssl],
                          start=False, stop=(tt == NST - 1), tile_position=tp,
                          skip_group_check=True)
        # assemble the [128,512] reciprocal pair (sliced-output pbcast is
        # broken, so: base-0 pbcasts + one ACT copy into the upper half),
        # letting the evictions run at full DVE width
        rb_t = work.tile([128, 512], F32, name=f"rbp_{j}", tag="rb", bufs=1)
        rb1 = work.tile([64, 512], F32, name=f"rb1_{j}", tag="rb1", bufs=1)
        G.partition_broadcast(rb_t[0:64, :], recip[0][:])
        G.partition_broadcast(rb1[:], recip[1][:])
        A.activation(rb_t[64:128, :], rb1[:], AF.Copy)
        V.tensor_mul(o_r[j][:, :], ps_or[:, :], rb_t[:])
        V.tensor_mul(o_i[j][:, :], ps_oi[:, :], rb_t[:])
        V.tensor_add(o_s[j][:, :], o_r[j][:, :], o_i[j][:, :])

    # Pair 0's scores (and their long DVE/ACT chain) are emitted before the
    # PE-dense V phase, so the elementwise engines ramp while the PE churns
    # through the V matmuls.
    qk_tiles = {}
    qk_tiles[0] = (qk_etile("q", 0), qk_etile("k", 0))
    qk_tiles[1] = (qk_etile("q", 1), qk_etile("k", 1))
    (qr, qi), (kr, ki, kn) = qk_tiles.pop(0)
    pending = (0, attention_scores(0, qr, qi, kr, ki, kn))
    load_wv(0)
    for st in range(NST):
        v_chunk(0, st)
    load_wv(1)
    for j in range(1, NP):
        if j - 1 < NST:
            v_chunk(1, j - 1)
        if j + 1 < NP:
            qk_tiles[j + 1] = (qk_etile("q", j + 1), qk_etile("k", j + 1))
        (qr, qi), (kr, ki, kn) = qk_tiles.pop(j)
        sc = attention_scores(j, qr, qi, kr, ki, kn)
        attention_av(pending[0], *pending[1])
        pending = (j, sc)
    attention_av(pending[0], *pending[1])

    ph12.release()

    # ---------------- phase 3: output projection (Karatsuba) -------------
    wo_pool = tc.alloc_tile_pool(name="wo_pool", bufs=1)
    out_pool = tc.alloc_tile_pool(name="out_pool", bufs=1)
    wo_sb = {}
    for nm in ("r", "i", "s"):
        t = wo_pool.tile([128, NK * DM], MM_DT, name=f"wo_{nm}_sb",
                         tag=f"wo{nm}", bufs=1)
        nc.sync.dma_start(t[:], io[f"wo_{nm}"][:])
        wo_sb[nm] = t

    def woch(nm, k, n):
        off = k * DM + n * 512
        return wo_sb[nm][:, off:off + 512]


    for st in range(NST):
        ssl = slice(st * 128, (st + 1) * 128)
        for n in range(2):
            p1 = psum.tile([128, 512], F32, name=f"pj1_{st}_{n}", tag="ps")
            p2 = psum.tile([128, 512], F32, name=f"pj2_{st}_{n}", tag="ps")
            p3 = psum.tile([128, 512], F32, name=f"pj3_{st}_{n}", tag="ps")
            for k in range(NK):
                PE.matmul(p1[:], o_r[k][:, ssl], woch("r", k, n),
                          start=(k == 0), stop=(k == NK - 1))
            for k in range(NK):
                PE.matmul(p2[:], o_i[k][:, ssl], woch("i", k, n),
                          start=(k == 0), stop=(k == NK - 1))
            for k in range(NK):
                PE.matmul(p3[:], o_s[k][:, ssl], woch("s", k, n),
                          start=(k == 0), stop=(k == NK - 1))
            to_r = out_pool.tile([128, 512], F32, name=f"otr_{st}_{n}",
                                 tag="out_r", bufs=2)
            to_i = out_pool.tile([128, 512], F32, name=f"oti_{st}_{n}",
                                 tag="out_i", bufs=2)
            ot = out_pool.tile([128, 512], F32, name=f"ott_{st}_{n}",
                               tag="out_t", bufs=2)
            A.activation(ot[:], p1[:], AF.Copy)
            V.tensor_sub(to_r[:], ot[:], p2[:])
            V.tensor_sub(to_i[:], p3[:], ot[:])
            V.tensor_sub(to_i[:], to_i[:], p2[:])
            nsl = slice(n * 512, (n + 1) * 512)
            nc.sync.dma_start(out[0, ssl, nsl], to_r[:])
            nc.sync.dma_start(out[1, ssl, nsl], to_i[:])

    out_pool.release()
    wo_pool.release()
    for p in (work, o_pool, v_pool, qk_pool, psum_sm, psum, const):
        p.release()


def _install_act_root():
    """Restrict walrus to the silu_and_others ACT table set (tanh+sin+abs
    +copy in ONE set) so the kernel never thrashes ACT_TABLE_LOADs.
    On any failure, degrade to the default tables (correct, slower)."""
    if os.environ.get("K_NO_ACTFIX"):
        return
    if os.environ.get("BASS_ACT_ROOT_JSON_PATH"):
        return
    try:
        _install_act_root_impl()
    except Exception:
        os.environ["K_NO_ACTFIX"] = "1"


def _install_act_root_impl():
    import json
    import tempfile
    from neuronxcc.driver.Job import Job
    from neuronxcc.driver.jobs.support.FindActInfo import findActInfoFile

    p = findActInfoFile(Job.getPackageDir(), "gen3")
    src_dir = os.path.dirname(p)
    with open(p) as f:
        d = json.load(f)
    d["act_func_sets"] = [e for e in d["act_func_sets"]
                          if e["name"] == "silu_and_others"]
    out_dir = tempfile.mkdtemp(prefix="act_silu_")
    for fn in os.listdir(src_dir):
        sp = os.path.join(src_dir, fn)
        if os.path.isfile(sp) and fn != "act_info.json":
            os.symlink(sp, os.path.join(out_dir, fn))
    with open(os.path.join(out_dir, "act_info.json"), "w") as f:
        json.dump(d, f)
    os.environ["BASS_ACT_ROOT_JSON_PATH"] = os.path.join(out_dir,
                                                         "act_info.json")
    # bass pre-places InstLoadActFuncSet ids indexing this same list; keep
    # the bass-side table view consistent with the trimmed act_info.json.
    import concourse.hw_specs as hw_specs
    import concourse.bacc as bacc_mod

    orig = hw_specs.get_activation_tables.__wrapped__

    @__import__("functools").cache
    def only_silu(arch):
        full = orig(arch)
        return {"silu_and_others": full["silu_and_others"]}

    hw_specs.get_activation_tables = only_silu
    bacc_mod.get_activation_tables = only_silu


def build_nc():
    _install_act_root()
    nc = bacc.Bacc("TRN2", target_bir_lowering=False, debug=False,
                   enable_asserts=False, num_devices=8)
    io = {}

    def inp(name, shape, dt=MM_DT):
        io[name] = nc.dram_tensor(name, shape, dt, kind="ExternalInput").ap()

    inp("x_r", [128, NK * S])
    inp("x_i", [128, NK * S])
    inp("x_s", [128, NK * S])
    inp("wq_r", [NQK, 128, NK * 128])
    inp("wq_i", [NQK, 128, NK * 128])
    inp("wq_s", [NQK, 128, NK * 128])
    inp("wv_r", [2, 128, NK * 512])
    inp("wv_i", [2, 128, NK * 512])
    inp("wv_s", [2, 128, NK * 512])
    inp("wo_r", [128, NK * DM])
    inp("wo_i", [128, NK * DM])
    inp("wo_s", [128, NK * DM])
    inp("cos", [128, S], TAB_DT)
    inp("sin", [128, S], TAB_DT)
    inp("cos_q", [128, S], TAB_DT)
    inp("sin_q", [128, S], TAB_DT)
    io["out"] = nc.dram_tensor("out", [2, S, DM], F32,
                               kind="ExternalOutput").ap()

    with tile.TileContext(nc) as tc:
        build_body(nc, tc, io)
    nc.compile()
    return nc


def host_inputs(xr, xi, wqkv_r, wqkv_i, wo_r, wo_i):
    """Pack full f32 inputs into 8 per-core in_maps."""
    np_mm = mybir.dt.np(MM_DT)
    np_tab = mybir.dt.np(TAB_DT)

    def pack_qk(w):  # (D, 3D) -> [16e][128p][8k*128]
        return np.ascontiguousarray(
            w[:, :2 * DM].reshape(NK, 128, NQK, 128).transpose(2, 1, 0, 3)
            .reshape(NQK, 128, NK * 128))

    def pack_v(w):  # -> [2n][128p][8k*512]
        return np.ascontiguousarray(
            w[:, 2 * DM:].reshape(NK, 128, 2, 512).transpose(2, 1, 0, 3)
            .reshape(2, 128, NK * 512))

    def pack_p(w):  # (NK,128,F) row-major -> [128p][NK*F]
        return np.ascontiguousarray(
            w.transpose(1, 0, 2).reshape(128, -1))

    wqkvT_r = np.ascontiguousarray(wqkv_r.T).astype(np_mm)  # (D, 3D)
    wqkvT_i = np.ascontiguousarray(wqkv_i.T).astype(np_mm)
    wqkvT_s = (wqkvT_r.astype(np.float32)
               + wqkvT_i.astype(np.float32)).astype(np_mm)
    woT_r = np.ascontiguousarray(wo_r.T.astype(np_mm))
    woT_i = np.ascontiguousarray(wo_i.T.astype(np_mm))
    woT_s = (woT_r.astype(np.float32)
             + woT_i.astype(np.float32)).astype(np_mm)

    inv_freq = 1.0 / (10000.0 ** (np.arange(DH, dtype=np.float64) / DH))
    ang = np.arange(S, dtype=np.float64)[:, None] * inv_freq[None, :]  # (S,Dh)
    cosT = np.cos(ang).T  # (Dh, S)
    sinT = np.sin(ang).T
    cos_t = np.ascontiguousarray(
        np.concatenate([cosT, cosT], axis=0)).astype(np_tab)  # (128, S)
    sin_t = np.ascontiguousarray(
        np.concatenate([sinT, sinT], axis=0)).astype(np_tab)

    shared = {
        "wq_r": pack_qk(wqkvT_r), "wq_i": pack_qk(wqkvT_i),
        "wq_s": pack_qk(wqkvT_s),
        "wv_r": pack_v(wqkvT_r), "wv_i": pack_v(wqkvT_i),
        "wv_s": pack_v(wqkvT_s),
        "wo_r": pack_p(woT_r.reshape(NK, 128, DM)),
        "wo_i": pack_p(woT_i.reshape(NK, 128, DM)),
        "wo_s": pack_p(woT_s.reshape(NK, 128, DM)),
        "cos": cos_t, "sin": sin_t,
        "cos_q": np.ascontiguousarray(
            np.concatenate([cosT, cosT], axis=0) * SCALE).astype(np_tab),
        "sin_q": np.ascontiguousarray(
            np.concatenate([sinT, sinT], axis=0) * SCALE).astype(np_tab),
    }
    in_maps = []
    for b in range(B):
        xT_r = xr[b].T.astype(np_mm).reshape(NK, 128, S)
        xT_i = xi[b].T.astype(np_mm).reshape(NK, 128, S)
        xT_s = (xT_r.astype(np.float32)
                + xT_i.astype(np.float32)).astype(np_mm)
        m = {"x_r": pack_p(xT_r), "x_i": pack_p(xT_i), "x_s": pack_p(xT_s)}
        m.update(shared)
        in_maps.append(m)
    return in_maps


_NC_CACHE = None


def get_nc():
    global _NC_CACHE
    if _NC_CACHE is None:
        _NC_CACHE = build_nc()
    return _NC_CACHE


def kernel(xr, xi, wqkv_r, wqkv_i, wo_r, wo_i):
    from concourse.bass_utils import run_bass_kernel_spmd

    _install_act_root()
    in_maps = host_inputs(np.asarray(xr, np.float32),
                          np.asarray(xi, np.float32),
                          np.asarray(wqkv_r, np.float32),
                          np.asarray(wqkv_i, np.float32),
                          np.asarray(wo_r, np.float32),
                          np.asarray(wo_i, np.float32))
    nc = get_nc()
    res = run_bass_kernel_spmd(nc, in_maps, core_ids=list(range(B)),
                               trace=bool(int(os.environ.get("K_TRACE", "0"))))
    out_r = np.stack([res.results[b]["out"][0] for b in range(B)])
    out_i = np.stack([res.results[b]["out"][1] for b in range(B)])
    kernel.last_results = res
    return out_r, out_i
